# revision 1
# baseline (speedup 1.0000x reference)
"""Trainium2 Bass kernel: cosine-similarity softmin retrieval (DSDM).

reference:  qn = q/||q||; an = a/||a||; sims = qn @ an^T            [B, N]
            w = softmax(10*sims) over N  (softmin of (1-sims)/0.1)
            out = (w @ A)                                           [B, D]

Strategy (8 NeuronCores, flash-attention-style split over N):
  - addresses [200000, 512] sharded row-wise, 25000 rows/core.
  - each core streams its shard once in 128-row tiles (bf16 on-chip, cast
    during the load DMA):
      * row norms ss = sum(a^2) on DVE (affine_mul_reduce)
      * 10/||a|| = exp(-0.5*ln(ss + eps) + ln10) on ACT (one table set)
      * A^T chunks via HWDGE xbar DMA-transpose (bf16, SBUF->SBUF) -- frees
        the PE from 2 of its 3 passes over A and avoids a PSUM->SBUF copy
      * s_raw^T [128j, 64b] = A_chunk @ qn^T via 4 PSUM-accumulated matmuls
      * w^T = Exp(s_raw^T * (10/||a||) - 10) on ACT (fixed shift: cos<=1,
        so logit-10 <= 0; no running max needed)
      * acc [64, 512] += w^T.T @ A in PSUM across all tiles
      * wsum [128, 64] += w^T on GPSIMD; ones-matmul partition-reduce at end
  - host: out = sum_c acc_c / sum_c l_c   (gather/unshard + tiny divide)

Padding: per-core row count 25000 = 195*128 + 40; the last tile's 88 pad
rows are zeroed and get exp bias -40 (weight ~4e-18, exactly negligible).
"""

import math
import os
from collections import OrderedDict

import numpy as np

import concourse.bass as bass
import concourse.tile as tile
from concourse import bacc, mybir
from concourse.bass_utils import run_bass_kernel_spmd
from concourse.masks import make_identity

DT = mybir.dt
AF = mybir.ActivationFunctionType
ALU = mybir.AluOpType

B = 64
D = 512
N_FULL = 200000
NCORES = 8
NPC = N_FULL // NCORES  # 25000
P = 128
G = 4  # tiles per DMA slab
LN10 = math.log(10.0)

# "pe" or "dma": how A^T chunks are produced
TRANSPOSE_MODE = os.environ.get("KERNEL_TRANSPOSE", "pe")
NORMS_MODE = os.environ.get("KERNEL_NORMS", "mixed")
NORM_DVE_OF8 = int(os.environ.get("KERNEL_NORM_DVE_OF8", "4"))  # tiles/8 on DVE
WSUM_MODE = os.environ.get("KERNEL_WSUM", "gpsimd")
SIMS_MODE = os.environ.get("KERNEL_SIMS", "quad")

LAST_RESULTS = None  # test harness reads exec_time_ns from here


def _patch_act_tables():
    """Prefer the combined natural_log_exp set so Ln/Exp/Square/Copy share
    one ACT table load instead of thrashing 2 loads per slab (~2.7us each)."""
    if getattr(bacc.get_activation_tables, "_patched", False):
        return
    orig = bacc.get_activation_tables

    keep = {AF.Ln, AF.Exp, AF.Square}

    def patched(arch):
        tabs = orig(arch)
        out = OrderedDict()
        for k, fns in tabs.items():
            if k == "natural_log_exp_and_others":
                out[k] = fns
            else:
                out[k] = {f for f in fns if f not in keep}
        return out

    patched._patched = True
    bacc.get_activation_tables = patched


def _build(npc=NPC):
    _patch_act_tables()
    ntiles = (npc + P - 1) // P
    G = max(g for g in range(1, 17) if ntiles % g == 0)  # tiles per slab
    nslabs = ntiles // G
    real_last = npc - (ntiles - 1) * P  # rows in final tile

    nc = bacc.Bacc("TRN2")
    q_d = nc.dram_tensor("query", [B, D], DT.float32, kind="ExternalInput")
    a_d = nc.dram_tensor("addresses", [npc, D], DT.float32, kind="ExternalInput")
    acc_d = nc.dram_tensor("acc", [B, D], DT.float32, kind="ExternalOutput")
    lsum_d = nc.dram_tensor("lsum", [B, 1], DT.float32, kind="ExternalOutput")

    with tile.TileContext(nc) as tc:
        with (
            tc.tile_pool(name="const", bufs=1) as const,
            tc.tile_pool(name="slab", bufs=4) as slab_pool,
            tc.tile_pool(name="at", bufs=8) as at_pool,
            tc.tile_pool(name="wt", bufs=4) as wt_pool,
            tc.tile_pool(name="small", bufs=4) as small,
            tc.tile_pool(name="ps_at", bufs=2, space="PSUM") as ps_at,
            tc.tile_pool(name="ps_s", bufs=2, space="PSUM") as ps_s,
            tc.tile_pool(name="ps_wt", bufs=2, space="PSUM") as ps_wt,
            tc.tile_pool(name="ps_one", bufs=1, space="PSUM") as ps_one,
            tc.tile_pool(name="ps_acc", bufs=1, space="PSUM") as ps_acc,
            tc.tile_pool(name="dram", bufs=1, space="DRAM") as dram_pool,
        ):
            ident = const.tile([P, P], DT.bfloat16)
            make_identity(nc, ident)
            bias_main = const.tile([P, 1], DT.float32)
            nc.vector.memset(bias_main, -10.0)
            bias_last = const.tile([P, 1], DT.float32)
            nc.vector.memset(bias_last, -40.0)
            if real_last > 0:
                nc.vector.memset(bias_last[:real_last], -10.0)
            ones = const.tile([P, 1], DT.float32)
            nc.vector.memset(ones, 1.0)
            eps12 = const.tile([P, 1], DT.float32)
            nc.vector.memset(eps12, 1e-12)
            ln10b = const.tile([P, 1], DT.float32)
            nc.vector.memset(ln10b, LN10)
            wsum = const.tile([P, B], DT.float32)
            nc.vector.memset(wsum, 0.0)
            wsum4 = const.tile([P, 4, B], DT.float32)
            nc.vector.memset(wsum4, 0.0)
            identf = const.tile([P, P], DT.float32)
            make_identity(nc, identf)

            # ---- query preprocessing: qn^T bf16 chunks [128d, 4c, 64b] ----
            q_sb = const.tile([B, D], DT.float32)
            nc.sync.dma_start(out=q_sb, in_=q_d[:, :])
            qsq = const.tile([B, D], DT.float32)
            ssq = const.tile([B, 1], DT.float32)
            nc.scalar.activation(qsq, q_sb, AF.Square, accum_out=ssq)
            lnq = const.tile([B, 1], DT.float32)
            nc.scalar.activation(lnq, ssq, AF.Ln, bias=eps12[:B])
            invq = const.tile([B, 1], DT.float32)
            nc.scalar.activation(invq, lnq, AF.Exp, scale=-0.5)
            qn = const.tile([B, D], DT.bfloat16)
            nc.vector.tensor_scalar_mul(out=qn, in0=q_sb, scalar1=invq)
            qnT = const.tile([P, 4, B], DT.bfloat16)
            for c in range(4):
                qt_ps = ps_one.tile([P, B], DT.bfloat16, tag="onebank")
                nc.tensor.transpose(qt_ps, qn[:, c * P:(c + 1) * P], ident[:B, :B])
                nc.scalar.copy(qnT[:, c, :], qt_ps)

            # ---- main streaming loop ----
            acc_ps = ps_acc.tile([B, D], DT.float32)
            nquads = ntiles // 4
            assert SIMS_MODE == "tile" or nquads * 4 == ntiles
            scr = dram_pool.tile([1, ntiles * P], DT.float32)
            slab_tiles = {}
            slab_inv = {}

            def ensure_slab(g):
                if g in slab_tiles:
                    return slab_tiles[g]
                a_sl = slab_pool.tile([P, G, D], DT.bfloat16)
                last_slab = g == nslabs - 1
                if not last_slab or real_last == P:
                    nc.gpsimd.dma_start(
                        out=a_sl,
                        in_=a_d[g * G * P:(g + 1) * G * P, :].rearrange(
                            "(t p) d -> p t d", p=P))
                else:
                    for t in range(G - 1):
                        r0 = (g * G + t) * P
                        nc.gpsimd.dma_start(out=a_sl[:, t, :], in_=a_d[r0:r0 + P, :])
                    nc.gpsimd.memset(a_sl[:, G - 1, :], 0)
                    nc.gpsimd.dma_start(
                        out=a_sl[:real_last, G - 1, :],
                        in_=a_d[(ntiles - 1) * P:npc, :])
                slab_tiles[g] = a_sl
                # norms for the slab + 10/||a|| + transposed flat copy to DRAM
                ss = small.tile([P, G], DT.float32, tag="ss")
                for t in range(G):
                    gt0 = g * G + t
                    sq = small.tile([P, D], DT.bfloat16, tag="sq")
                    if (gt0 % 8) < NORM_DVE_OF8:
                        nc.vector.affine_mul_reduce(
                            out=sq, accum_out=ss[:, t:t + 1],
                            in0=a_sl[:, t, :], in1=a_sl[:, t, :], scale=1.0,
                            bias=0.0)
                    else:
                        nc.scalar.activation(sq, a_sl[:, t, :], AF.Square,
                                             accum_out=ss[:, t:t + 1])
                lns = small.tile([P, G], DT.float32, tag="lns")
                nc.scalar.activation(lns, ss, AF.Ln, bias=eps12)
                inv = small.tile([P, G], DT.float32, tag="inv")
                nc.scalar.activation(inv, lns, AF.Exp, scale=-0.5, bias=ln10b)
                slab_inv[g] = inv
                if SIMS_MODE == "quad":
                    ivt_ps = ps_one.tile([G, P], DT.float32, tag="onebank")
                    nc.tensor.transpose(ivt_ps, inv, identf)
                    ivt = small.tile([G, P], DT.float32, tag="ivt_sb")
                    nc.vector.tensor_copy(ivt, ivt_ps)
                    nc.sync.dma_start(out=a_scr_view(g), in_=ivt)
                return a_sl

            def a_scr_view(g):
                return bass.AP(
                    tensor=scr.tensor, offset=scr.offset + g * G * P,
                    ap=[[P, G], [1, P]])

            def a_tile(gt):
                g, t = divmod(gt, G)
                return ensure_slab(g)[:, t, :]

            if SIMS_MODE == "quad":
                pending = None  # (q, w_q) awaiting back stage

                def stage_front(q):
                    at_tiles = []
                    for t in range(4):
                        gt = 4 * q + t
                        a_t = a_tile(gt)
                        at_sb = at_pool.tile([P, 4, P], DT.bfloat16)
                        at_ps = ps_at.tile([P, 4, P], DT.bfloat16)
                        for c in range(4):
                            nc.tensor.transpose(
                                at_ps[:, c, :], a_t[:, c * P:(c + 1) * P], ident)
                        nc.vector.tensor_copy(at_sb, at_ps)
                        at_tiles.append(at_sb)
                    inv_bc = wt_pool.tile([B, 4 * P], DT.float32, tag="inv_bc")
                    nc.gpsimd.dma_start(
                        out=inv_bc,
                        in_=bass.AP(tensor=scr.tensor,
                                    offset=scr.offset + q * 4 * P,
                                    ap=[[0, B], [1, 4 * P]]))
                    s_ps = ps_s.tile([B, 4 * P], DT.float32, tag="s")
                    for t in range(4):
                        for c in range(4):
                            nc.tensor.matmul(
                                s_ps[:, t * P:(t + 1) * P],
                                lhsT=qnT[:, c, :], rhs=at_tiles[t][:, c, :],
                                start=(c == 0), stop=(c == 3))
                    s_sc = wt_pool.tile([B, 4 * P], DT.float32, tag="s_sc")
                    nc.vector.tensor_mul(s_sc, s_ps, inv_bc)
                    w_q = wt_pool.tile([B, 4 * P], DT.bfloat16, tag="w_q")
                    nc.scalar.activation(w_q, s_sc, AF.Exp, bias=bias_main[:B])
                    return w_q

                def stage_back(q, w_q):
                    wt_ps = ps_wt.tile([P, 4, B], DT.bfloat16)
                    for t in range(4):
                        nc.tensor.transpose(
                            wt_ps[:, t, :], w_q[:, t * P:(t + 1) * P],
                            ident[:B, :B])
                    wt_sb = wt_pool.tile([P, 4, B], DT.bfloat16, tag="wt_sb")
                    nc.vector.tensor_copy(wt_sb, wt_ps)
                    for t in range(4):
                        gt = 4 * q + t
                        nc.tensor.matmul(
                            acc_ps, lhsT=wt_sb[:, t, :], rhs=a_tile(gt),
                            start=(gt == 0), stop=(gt == ntiles - 1))
                    nc.gpsimd.tensor_add(wsum4, wsum4, wt_sb)

                for q in range(nquads):
                    w_q = stage_front(q)
                    if pending is not None:
                        stage_back(*pending)
                    pending = (q, w_q)
                if pending is not None:
                    stage_back(*pending)
            else:
                for gt in range(ntiles):
                    g, t = divmod(gt, G)
                    a_sl = ensure_slab(g)
                    at_sb = at_pool.tile([P, 4, P], DT.bfloat16)
                    at_ps = ps_at.tile([P, 4, P], DT.bfloat16)
                    for c in range(4):
                        nc.tensor.transpose(
                            at_ps[:, c, :], a_sl[:, t, c * P:(c + 1) * P], ident)
                    nc.vector.tensor_copy(at_sb, at_ps)
                    s_ps = ps_s.tile([P, B], DT.float32, tag="s")
                    for c in range(4):
                        nc.tensor.matmul(
                            s_ps, lhsT=at_sb[:, c, :], rhs=qnT[:, c, :],
                            start=(c == 0), stop=(c == 3))
                    wt = wt_pool.tile([P, B], DT.bfloat16, tag="wt")
                    inv = slab_inv[g]
                    nc.scalar.activation(
                        wt, s_ps, AF.Exp,
                        bias=bias_last if gt == ntiles - 1 else bias_main,
                        scale=inv[:, t:t + 1])
                    nc.tensor.matmul(
                        acc_ps, lhsT=wt, rhs=a_sl[:, t, :],
                        start=(gt == 0), stop=(gt == ntiles - 1))
                    nc.gpsimd.tensor_add(wsum, wsum, wt)

            # ---- epilogue: normalizer + writeback ----
            l_ps = ps_one.tile([B, 1], DT.float32, tag="onebank")
            if SIMS_MODE == "quad":
                for t in range(4):
                    nc.tensor.matmul(l_ps, lhsT=wsum4[:, t, :], rhs=ones,
                                     start=(t == 0), stop=(t == 3))
            else:
                nc.tensor.matmul(l_ps, lhsT=wsum, rhs=ones)
            acc_sb = const.tile([B, D], DT.float32)
            nc.scalar.copy(acc_sb, acc_ps)
            l_sb = const.tile([B, 1], DT.float32)
            nc.vector.tensor_copy(l_sb, l_ps)
            nc.sync.dma_start(out=acc_d[:, :], in_=acc_sb)
            nc.sync.dma_start(out=lsum_d[:, :], in_=l_sb)

    nc.finalize()
    return nc


_NC_CACHE = {}


def _get_nc(npc=NPC):
    if npc not in _NC_CACHE:
        _NC_CACHE[npc] = _build(npc)
    return _NC_CACHE[npc]


def kernel(query, addresses):
    global LAST_RESULTS
    query = np.ascontiguousarray(np.asarray(query), dtype=np.float32)
    addresses = np.ascontiguousarray(np.asarray(addresses), dtype=np.float32)
    n = addresses.shape[0]
    npc = n // NCORES
    assert npc * NCORES == n
    nc = _get_nc(npc)
    in_maps = [
        {"query": query, "addresses": addresses[c * npc:(c + 1) * npc]}
        for c in range(NCORES)
    ]
    res = run_bass_kernel_spmd(nc, in_maps, core_ids=list(range(NCORES)))
    LAST_RESULTS = res
    acc = np.zeros((B, D), np.float64)
    l = np.zeros((B, 1), np.float64)
    ntiles = (npc + P - 1) // P
    n_pad = ntiles * P - npc  # zero rows in the padded last tile
    for r in res.results:
        acc += r["acc"].astype(np.float64)
        l += r["lsum"].astype(np.float64)
        if SIMS_MODE == "quad" and n_pad:
            # each pad row contributes exactly exp(0*scale - 10)
            l -= n_pad * math.exp(-10.0)
    return (acc / l).astype(np.float32)



# revision 4
# speedup vs baseline: 1.4422x; 1.4422x over previous
"""Trainium2 Bass kernel: cosine-similarity softmin retrieval (DSDM), v2.

reference:  qn = q/||q||; an = a/||a||; sims = qn @ an^T            [B, N]
            w = softmax(10*sims) over N  (softmin of (1-sims)/0.1)
            out = (w @ A)                                           [B, D]

v2 strategy (8 NeuronCores, flash-attention-style split over N):
  - addresses sharded row-wise, 25000 rows/core, host-padded to
    25088 = 128*196 zero rows so each PARTITION owns a contiguous
    blocked run of 196 rows (row = p*196 + t).  The slab DMA then reads
    G*2KB contiguous per partition -> big packets, ~line-rate HBM
    (the old interleaved layout produced 1.1KB packets at ~110 GB/s).
  - per 128-row tile t (bf16 on-chip, cast during the SWDGE load):
      * row norms ss = sum(a^2) on DVE/ACT (knob), 10/||a|| via Ln/Exp
      * A^T chunks via 4 PE transposes (PSUM) + engine copy to SBUF
      * s^T [128n, 64b] = sum_c at_c^T-stationary @ qnT_c (4 matmuls,
        64-col streams, PSUM-accumulated; FWL-eligible weights)
      * w^T = Exp(s^T * inv[:,t] - 10) on ACT (per-partition scale;
        fixed shift valid since cos<=1)
      * accT [128d, 4c, 64b] += a_chunk-stationary @ w^T (4 matmuls,
        64-col streams) -- output transposed so streams stay short
      * wsum4 += w^T per quad on GPSIMD; ones-matmul reduce at end
  - host: out = (sum_c accT_c).T / sum_c l_c, minus the exact
    88*exp(-10) pad-row contribution per core.
"""

import math
import os
from collections import OrderedDict

import numpy as np

import concourse.bass as bass
import concourse.tile as tile
from concourse import bacc, mybir
from concourse.bass_utils import run_bass_kernel_spmd
from concourse.masks import make_identity

DT = mybir.dt
AF = mybir.ActivationFunctionType

B = 64
D = 512
N_FULL = 200000
NCORES = 8
NPC = N_FULL // NCORES  # 25000
P = 128
LN10 = math.log(10.0)

G_MAX = int(os.environ.get("KERNEL_G", "7"))  # tiles per DMA slab (max)
NORM_DVE_OF8 = int(os.environ.get("KERNEL_NORM_DVE_OF8", "5"))  # tiles/8 on DVE
COPY_ACT_OF8 = int(os.environ.get("KERNEL_COPY_ACT_OF8", "2"))  # copies/8 on ACT
SLAB_BUFS = int(os.environ.get("KERNEL_SLAB_BUFS", "4"))

LAST_RESULTS = None  # test harness reads exec_time_ns from here


def _patch_act_tables():
    """Prefer the combined natural_log_exp set so Ln/Exp/Square/Copy share
    one ACT table load instead of thrashing 2 loads per slab (~2.7us each)."""
    if getattr(bacc.get_activation_tables, "_patched", False):
        return
    orig = bacc.get_activation_tables

    keep = {AF.Ln, AF.Exp, AF.Square}

    def patched(arch):
        tabs = orig(arch)
        out = OrderedDict()
        for k, fns in tabs.items():
            if k == "natural_log_exp_and_others":
                out[k] = fns
            else:
                out[k] = {f for f in fns if f not in keep}
        return out

    patched._patched = True
    bacc.get_activation_tables = patched


def _build(npc_pad):
    _patch_act_tables()
    assert npc_pad % P == 0
    T = npc_pad // P  # rows per partition (= number of 128-row tiles)
    G = max(g for g in range(1, G_MAX + 1) if T % g == 0)
    nslabs = T // G

    nc = bacc.Bacc("TRN2")
    q_d = nc.dram_tensor("query", [B, D], DT.float32, kind="ExternalInput")
    a_d = nc.dram_tensor("addresses", [npc_pad, D], DT.float32,
                         kind="ExternalInput")
    acc_d = nc.dram_tensor("acc", [P, 4 * B], DT.float32, kind="ExternalOutput")
    lsum_d = nc.dram_tensor("lsum", [B, 1], DT.float32, kind="ExternalOutput")

    with tile.TileContext(nc) as tc:
        with (
            tc.tile_pool(name="const", bufs=1) as const,
            tc.tile_pool(name="slab", bufs=SLAB_BUFS) as slab_pool,
            tc.tile_pool(name="at", bufs=3) as at_pool,
            tc.tile_pool(name="wt", bufs=3) as wt_pool,
            tc.tile_pool(name="small", bufs=3) as small,
            tc.tile_pool(name="ps_at", bufs=2, space="PSUM") as ps_at,
            tc.tile_pool(name="ps_s", bufs=2, space="PSUM") as ps_s,
            tc.tile_pool(name="ps_one", bufs=1, space="PSUM") as ps_one,
            tc.tile_pool(name="ps_acc", bufs=1, space="PSUM") as ps_acc,
        ):
            ident = const.tile([P, P], DT.bfloat16)
            make_identity(nc, ident)
            bias_main = const.tile([P, 1], DT.float32)
            nc.vector.memset(bias_main, -10.0)
            ones = const.tile([P, 1], DT.float32)
            nc.vector.memset(ones, 1.0)
            eps12 = const.tile([P, 1], DT.float32)
            nc.vector.memset(eps12, 1e-12)
            ln10b = const.tile([P, 1], DT.float32)
            nc.vector.memset(ln10b, LN10)
            wsum4 = const.tile([P, 4, B], DT.float32)
            nc.vector.memset(wsum4, 0.0)

            # ---- query preprocessing: qn^T bf16 chunks [128d, 4c, 64b] ----
            q_sb = const.tile([B, D], DT.float32)
            nc.sync.dma_start(out=q_sb, in_=q_d[:, :])
            qsq = const.tile([B, D], DT.float32)
            ssq = const.tile([B, 1], DT.float32)
            nc.scalar.activation(qsq, q_sb, AF.Square, accum_out=ssq)
            lnq = const.tile([B, 1], DT.float32)
            nc.scalar.activation(lnq, ssq, AF.Ln, bias=eps12[:B])
            invq = const.tile([B, 1], DT.float32)
            nc.scalar.activation(invq, lnq, AF.Exp, scale=-0.5)
            qn = const.tile([B, D], DT.bfloat16)
            nc.vector.tensor_scalar_mul(out=qn, in0=q_sb, scalar1=invq)
            qnT = const.tile([P, 4, B], DT.bfloat16)
            for c in range(4):
                qt_ps = ps_one.tile([P, B], DT.bfloat16, tag="onebank")
                nc.tensor.transpose(qt_ps, qn[:, c * P:(c + 1) * P], ident[:B, :B])
                nc.scalar.copy(qnT[:, c, :], qt_ps)

            # ---- main streaming loop over tiles (blocked row layout) ----
            accT_ps = ps_acc.tile([P, 4, B], DT.float32)
            slab_tiles = {}
            slab_inv = {}

            def ensure_slab(g):
                """DMA slab g and compute its norms -> returns [P, G, D] tile."""
                if g in slab_tiles:
                    return slab_tiles[g]
                a_sl = slab_pool.tile([P, G, D], DT.bfloat16)
                # partition p reads rows p*T + [g*G, (g+1)*G) -- contiguous
                nc.gpsimd.dma_start(
                    out=a_sl,
                    in_=a_d[:, :].rearrange(
                        "(p t) d -> p t d", p=P)[:, g * G:(g + 1) * G, :])
                slab_tiles[g] = a_sl
                ss = small.tile([P, G], DT.float32, tag="ss")
                for t in range(G):
                    gt0 = g * G + t
                    if (gt0 % 8) < NORM_DVE_OF8:
                        sq = small.tile([P, D], DT.bfloat16, tag="sqd")
                        nc.vector.affine_mul_reduce(
                            out=sq, accum_out=ss[:, t:t + 1],
                            in0=a_sl[:, t, :], in1=a_sl[:, t, :], scale=1.0,
                            bias=0.0)
                    else:
                        sq = small.tile([P, D], DT.bfloat16, tag="sqa")
                        nc.scalar.activation(sq, a_sl[:, t, :], AF.Square,
                                             accum_out=ss[:, t:t + 1])
                lns = small.tile([P, G], DT.float32, tag="lns")
                nc.scalar.activation(lns, ss, AF.Ln, bias=eps12)
                inv = small.tile([P, G], DT.float32, tag="inv")
                nc.scalar.activation(inv, lns, AF.Exp, scale=-0.5, bias=ln10b)
                slab_inv[g] = inv
                return a_sl

            def a_tile(gt):
                g, t = divmod(gt, G)
                return ensure_slab(g)[:, t, :]

            at_tiles = {}  # gt -> at_sb [P, 4, P] (A^T chunks in SBUF)
            wt_quads = {}  # q -> wt_q [P, 4, B]

            def stage_front(gt):
                """PE transposes of tile gt + copy PSUM->SBUF."""
                a_t = a_tile(gt)
                at_ps = ps_at.tile([P, 4, P], DT.bfloat16)
                for c in range(4):
                    nc.tensor.transpose(
                        at_ps[:, c, :], a_t[:, c * P:(c + 1) * P], ident)
                at_sb = at_pool.tile([P, 4, P], DT.bfloat16)
                if (gt % 8) < COPY_ACT_OF8:
                    nc.scalar.copy(at_sb, at_ps)
                else:
                    nc.vector.tensor_copy(at_sb, at_ps)
                at_tiles[gt] = at_sb

            def stage_mid(gt):
                """sims matmuls (A^T stationary, qnT streamed) + exp."""
                at_sb = at_tiles.pop(gt)
                s_ps = ps_s.tile([P, B], DT.float32, tag="s")
                for c in range(4):
                    nc.tensor.matmul(
                        s_ps, lhsT=at_sb[:, c, :], rhs=qnT[:, c, :],
                        start=(c == 0), stop=(c == 3))
                q_idx, t4 = divmod(gt, 4)
                if t4 == 0:
                    wt_q = wt_pool.tile([P, 4, B], DT.bfloat16, tag="wt")
                    wt_quads[q_idx] = wt_q
                g, t = divmod(gt, G)
                inv = slab_inv[g]
                nc.scalar.activation(
                    wt_quads[q_idx][:, t4, :], s_ps, AF.Exp,
                    bias=bias_main, scale=inv[:, t:t + 1])

            def stage_back(gt, ntiles):
                """acc matmuls (A chunk stationary, w^T streamed) + wsum."""
                q_idx, t4 = divmod(gt, 4)
                wt_q = wt_quads[q_idx]
                a_t = a_tile(gt)
                for c in range(4):
                    nc.tensor.matmul(
                        accT_ps[:, c, :], lhsT=a_t[:, c * P:(c + 1) * P],
                        rhs=wt_q[:, t4, :],
                        start=(gt == 0), stop=(gt == ntiles - 1))
                if t4 == 3 or gt == ntiles - 1:
                    nc.gpsimd.tensor_add(wsum4, wsum4, wt_q)
                    del wt_quads[q_idx]

            for gt in range(T):
                stage_front(gt)
                if gt >= 1:
                    stage_mid(gt - 1)
                if gt >= 2:
                    stage_back(gt - 2, T)
            stage_mid(T - 1)
            stage_back(T - 2, T)
            stage_back(T - 1, T)

            # ---- epilogue: normalizer + writeback ----
            l_ps = ps_one.tile([B, 1], DT.float32, tag="onebank")
            for t in range(4):
                nc.tensor.matmul(l_ps, lhsT=wsum4[:, t, :], rhs=ones,
                                 start=(t == 0), stop=(t == 3))
            acc_sb = const.tile([P, 4, B], DT.float32)
            nc.vector.tensor_copy(acc_sb, accT_ps)
            l_sb = const.tile([B, 1], DT.float32)
            nc.vector.tensor_copy(l_sb, l_ps)
            nc.sync.dma_start(out=acc_d[:, :], in_=acc_sb)
            nc.sync.dma_start(out=lsum_d[:, :], in_=l_sb)

    nc.finalize()
    return nc


_NC_CACHE = {}


def _get_nc(npc_pad):
    if npc_pad not in _NC_CACHE:
        _NC_CACHE[npc_pad] = _build(npc_pad)
    return _NC_CACHE[npc_pad]


def kernel(query, addresses):
    global LAST_RESULTS
    query = np.ascontiguousarray(np.asarray(query), dtype=np.float32)
    addresses = np.ascontiguousarray(np.asarray(addresses), dtype=np.float32)
    n = addresses.shape[0]
    npc = n // NCORES
    assert npc * NCORES == n
    npc_pad = ((npc + P - 1) // P) * P
    n_pad = npc_pad - npc  # zero pad rows per core
    nc = _get_nc(npc_pad)
    in_maps = []
    for c in range(NCORES):
        shard = addresses[c * npc:(c + 1) * npc]
        if n_pad:
            shard = np.concatenate(
                [shard, np.zeros((n_pad, D), np.float32)], axis=0)
        in_maps.append({"query": query, "addresses": shard})
    res = run_bass_kernel_spmd(nc, in_maps, core_ids=list(range(NCORES)))
    LAST_RESULTS = res
    acc = np.zeros((B, D), np.float64)
    l = np.zeros((B, 1), np.float64)
    for r in res.results:
        # accT [128, 4, 64]: value = accT[dl, c, b] -> acc[b, c*128+dl]
        accT = r["acc"].astype(np.float64).reshape(P, 4, B)
        acc += accT.transpose(2, 1, 0).reshape(B, D)
        l += r["lsum"].astype(np.float64)
        # each zero pad row contributes exactly exp(0*scale - 10)
        l -= n_pad * math.exp(-10.0)
    return (acc / l).astype(np.float32)
